# revision 32
# baseline (speedup 1.0000x reference)
"""Trainium2 Bass kernel for EnhancedCrossAttention3D.

Computes, per batch b:
    q = Wq @ x1 + bq            (x1 = branch1[b] reshaped [C, N])
    k = Wk @ x2 + bk
    v = Wv @ x2 + bv
    attn = softmax((q^T k) / sqrt(C), axis=keys)
    out = Wp @ (attn @ v^T)^T + bp      -> [C, N]

Sharding: 8 cores = 2 batches x 4 query shards of 2048. Each core gets its
full K/V source (branch2[b]) and its query shard; no collectives.

On-core algorithm (flash-style, S^T layout):
    S^T[m, n] = sum_c k[c, m] * qT[c, n]   (m = key index on partitions)
    E = exp(S^T / 8)                       (logits are tiny; no max-sub needed)
    PV[c, n]  = sum_m [v | 1][m, c] * E[m, n]   -> row 64 is the softmax denom
    out[o, n] = Wp @ (PV[0:64] / denom + bv) + bp
(bv is folded in after normalization: attn rows sum to 1.)

Structure notes (this PE is clocked at 1.2 GHz; ~1.35us of exp per key chunk
on the Scalar engine is the pacing budget):
  * queries run in two 1024-wide half-passes so the PV accumulator needs only
    2 PSUM banks, freeing 6 banks for a triple-buffered S^T pipeline;
  * S^T matmuls are row-packed: k and qT live in both partition halves and
    two K=64 matmuls run concurrently in opposite halves of the PE array;
  * S/PV matmul operands are bf16 (full-rate streaming + fast weight load),
    PSUM accumulation stays fp32;
  * each pass's epilogue first evacuates the accumulator to SBUF (so the next
    pass can reuse the banks ~1us later) and its PE work is emitted a few
    chunks into the next pass to keep the in-order PE queue from stalling.
"""

import numpy as np
from contextlib import ExitStack

import concourse.bass as bass
import concourse.mybir as mybir
import concourse.tile as tile
from concourse import bacc
from concourse.bass import ts
from concourse.bass_utils import run_bass_kernel_spmd

B, C, D, H, W = 2, 64, 8, 32, 32
N = D * H * W              # 8192 keys per batch
NCORES = 8
QSH = (B * N) // NCORES    # 2048 queries per core
MCH = N // 128             # 64 key chunks of 128
NH = QSH // 1024           # 2 query half-passes
F32 = mybir.dt.float32
F32R = mybir.dt.float32r
BF16 = mybir.dt.bfloat16
AF = mybir.ActivationFunctionType
ALU = mybir.AluOpType

_CACHE = {}


def _emit(tc, xq, xkv2, wq, wk, wv, wp, bq, bk, bv, bp, out):
    nc = tc.nc
    ctx = ExitStack()
    # bf16/f32r writes below intentionally round fp32; errors wash out in the
    # 8192-term attention sums and sit ~1e-4 of output scale.
    ctx.enter_context(nc.allow_low_precision(reason="bf16 attention operands"))
    const = ctx.enter_context(tc.tile_pool(name="const", bufs=1))
    big = ctx.enter_context(tc.tile_pool(name="big", bufs=1))
    ps3 = ctx.enter_context(tc.tile_pool(name="ps3", bufs=3, space="PSUM"))
    acc_p = ctx.enter_context(tc.tile_pool(name="acc", bufs=1, space="PSUM"))
    ex_pool = ctx.enter_context(tc.tile_pool(name="ex", bufs=12))
    small = ctx.enter_context(tc.tile_pool(name="small", bufs=2))

    # ---- loads (gpsimd DMAs cast f32 -> bf16 in flight) ----
    xq_bf = big.tile([C, QSH], BF16)
    for s in range(2):
        nc.gpsimd.dma_start(out=xq_bf[:, ts(s, QSH // 2)],
                            in_=xq[:, ts(s, QSH // 2)])
    xkv_bf = big.tile([C, N], BF16)
    for s in range(8):
        nc.gpsimd.dma_start(out=xkv_bf[:, ts(s, N // 8)],
                            in_=xkv2[:, ts(s, N // 8)])
    # weights arrive pre-transposed from the host (contiguous DMAs), f32 on
    # the otherwise-idle sync queue, tiny casts on DVE
    wqT_f = const.tile([C, C], F32)
    nc.sync.dma_start(out=wqT_f, in_=wq)
    wqT = const.tile([C, C], BF16)
    nc.vector.tensor_copy(wqT, wqT_f)
    wkT_f = const.tile([C, C], F32)
    nc.sync.dma_start(out=wkT_f, in_=wk)
    wkT = const.tile([C, C], BF16)
    nc.vector.tensor_copy(wkT, wkT_f)
    wvT_f = const.tile([C, C], F32)
    nc.sync.dma_start(out=wvT_f, in_=wv)
    wvT = const.tile([C, C], BF16)
    nc.vector.tensor_copy(wvT, wvT_f)
    # projection weight with bp as a 65th row: out = [Wp^T; bp]^T @ [pvn; 1]
    wpT = const.tile([C + 1, C], F32R)
    nc.sync.dma_start(out=wpT[0:C, :], in_=wp)
    nc.sync.dma_start(out=wpT[C:C + 1, :],
                      in_=bp.rearrange("(one c) -> one c", one=1))
    bq_sb = const.tile([C, 1], F32)
    nc.sync.dma_start(out=bq_sb, in_=bq.rearrange("(c one) -> c one", one=1))
    bk_sb = const.tile([C, 1], F32)
    nc.sync.dma_start(out=bk_sb, in_=bk.rearrange("(c one) -> c one", one=1))
    bv_sb = const.tile([C, 1], F32)
    nc.sync.dma_start(out=bv_sb, in_=bv.rearrange("(c one) -> c one", one=1))
    ones_f32 = const.tile([128, MCH], F32)
    nc.vector.memset(ones_f32, 1.0)

    # ---- q/k projections ----
    # Both live twice (partitions 0-63 and 64-127) so the S^T matmuls can be
    # row-packed; bias-add on DVE (lower half), bf16 fast-copy for the upper.
    qT_sb = big.tile([128, QSH], BF16)
    k_sb = big.tile([128, N], BF16)
    NKT = N // 512

    def q_proj(t):
        pq = ps3.tile([128, 1024], F32, tag="ps")
        nc.tensor.matmul(pq[0:C, 0:512], lhsT=wqT, rhs=xq_bf[:, ts(t, 512)],
                         start=True, stop=True)
        nc.vector.tensor_scalar_add(qT_sb[0:C, ts(t, 512)], pq[0:C, 0:512],
                                    bq_sb)
        nc.vector.tensor_copy(qT_sb[C:2 * C, ts(t, 512)],
                              qT_sb[0:C, ts(t, 512)])

    def k_proj(t):
        # projects k columns t*512..(t+1)*512 (covers key chunks 4t..4t+3)
        pk = ps3.tile([128, 1024], F32, tag="ps")
        nc.tensor.matmul(pk[0:C, 0:512], lhsT=wkT, rhs=xkv_bf[:, ts(t, 512)],
                         start=True, stop=True)
        nc.vector.tensor_scalar_add(k_sb[0:C, ts(t, 512)], pk[0:C, 0:512],
                                    bk_sb)
        nc.vector.tensor_copy(k_sb[C:2 * C, ts(t, 512)], k_sb[0:C, ts(t, 512)])

    # pass 0 needs q tiles 0-1 and k tile 0 before S(0) can go
    q_proj(0)
    q_proj(1)
    k_proj(0)
    q_proj(2)
    q_proj(3)
    k_proj(1)
    k_done = 2
    # v[m, c] with a ones column (row 64 of PV becomes the softmax denominator)
    v_sb = big.tile([128, MCH, C + 1], BF16)
    nc.vector.tensor_copy(v_sb[:, :, C], ones_f32)

    def epilogue_dve(acc):
        """Normalize the accumulator (+bv) off-PE; returns the pvn tiles."""
        denom = small.tile([1, 1024], F32, tag="denom")
        nc.vector.tensor_copy(denom, acc[C:C + 1, :])
        rbd = small.tile([C, 1024], F32, tag="rbd")
        nc.gpsimd.partition_broadcast(rbd, denom)
        rb = small.tile([C, 1024], F32, tag="rb")
        nc.vector.reciprocal(rb, rbd)
        pvns = []
        for t in range(2):
            pvn = small.tile([C + 1, 512], F32R, tag="pvn")
            nc.vector.memset(pvn.bitcast(F32)[C:C + 1, :], 1.0)
            nc.vector.tensor_mul(pvn[0:C, :], acc[0:C, ts(t, 512)],
                                 rb[:, ts(t, 512)])
            nc.vector.tensor_scalar_add(pvn[0:C, :], pvn[0:C, :], bv_sb)
            pvns.append(pvn)
        return pvns

    def epilogue_pe(p, pvns):
        """Project the normalized PV and store out[:, p*1024:(p+1)*1024]."""
        for t in range(2):
            pp = ps3.tile([128, 1024], F32, tag="ps")
            nc.tensor.matmul(pp[0:C, 0:512], lhsT=wpT, rhs=pvns[t],
                             start=True, stop=True)
            o_sb = small.tile([C, 512], F32, tag="o")
            nc.vector.tensor_copy(o_sb, pp[0:C, 0:512])
            nc.sync.dma_start(out=out[:, ts(p * 2 + t, 512)], in_=o_sb)

    # ---- flash loop: two query half-passes over all key chunks ----
    # PV trails S/exp by LAG chunks (ex tiles buffer the gap) so that at a
    # pass boundary the next pass's S/exp pipeline runs while the previous
    # accumulator is still being normalized (PV's WAR wait then doesn't
    # stall the in-order PE queue).
    LAG = 10
    pending = None
    exq = {}
    acc = None
    for u in range(NH * MCH + LAG):
        if u < NH * MCH:
            p, i = divmod(u, MCH)
            if p == 0 and i % 4 == 0 and k_done < NKT:
                # produce k two tiles (8 chunks) ahead of consumption
                k_proj(k_done)
                k_done += 1
            if p == 0 and i % 4 == 0:
                # v chunks i..i+3: 4 matmuls into one psum slot, 1 cast
                g = i // 4
                pv = ps3.tile([128, 1024], F32, tag="ps")
                for j in range(4):
                    nc.tensor.matmul(pv[:, ts(j, C)],
                                     lhsT=xkv_bf[:, ts(4 * g + j, 128)],
                                     rhs=wvT, start=True, stop=True)
                nc.vector.tensor_copy(
                    v_sb[:, 4 * g:4 * g + 4, 0:C],
                    pv[:, 0:4 * C].rearrange("p (g c) -> p g c", c=C))
            if pending is not None and u == MCH + 14:
                epilogue_pe(*pending)
                pending = None
            s_ps = ps3.tile([128, 1024], F32, tag="ps")
            for sub in range(2):
                lo = C * sub
                nc.tensor.matmul(s_ps[:, ts(sub, 512)],
                                 lhsT=k_sb[lo:lo + C, ts(i, 128)],
                                 rhs=qT_sb[lo:lo + C,
                                           p * 1024 + 512 * sub:
                                           p * 1024 + 512 * (sub + 1)],
                                 start=True, stop=True)
            ex = ex_pool.tile([128, 1024], BF16)
            nc.scalar.activation(ex, s_ps, AF.Exp, scale=0.125)
            exq[u] = ex
        j = u - LAG
        if j >= 0 and j < NH * MCH:
            jp, ji = divmod(j, MCH)
            if ji == 0:
                acc = acc_p.tile([C + 1, 1024], F32, tag="acc")
            ex = exq.pop(j)
            for sub in range(2):
                nc.tensor.matmul(acc[:, ts(sub, 512)], lhsT=v_sb[:, ji, :],
                                 rhs=ex[:, ts(sub, 512)],
                                 start=(ji == 0), stop=(ji == MCH - 1),
                                 skip_group_check=True)
            if ji == MCH - 1:
                pvns = epilogue_dve(acc)
                if jp < NH - 1:
                    pending = (jp, pvns)
                else:
                    epilogue_pe(jp, pvns)
    ctx.close()


def _build():
    nc = bacc.Bacc("TRN2", target_bir_lowering=False, debug=False,
                   num_devices=NCORES)
    aps = {}
    aps["xq"] = nc.dram_tensor("xq", [C, QSH], F32, kind="ExternalInput").ap()
    aps["xkv2"] = nc.dram_tensor("xkv2", [C, N], F32, kind="ExternalInput").ap()
    for nm in ("wq", "wk", "wv"):
        aps[nm] = nc.dram_tensor(nm, [C, C], F32, kind="ExternalInput").ap()
    aps["wp"] = nc.dram_tensor("wp", [C, C], F32R, kind="ExternalInput").ap()
    for nm in ("bq", "bk", "bv"):
        aps[nm] = nc.dram_tensor(nm, [C], F32, kind="ExternalInput").ap()
    aps["bp"] = nc.dram_tensor("bp", [C], F32R, kind="ExternalInput").ap()
    aps["out"] = nc.dram_tensor("out", [C, QSH], F32, kind="ExternalOutput").ap()
    with tile.TileContext(nc) as tc:
        _emit(tc, **aps)
    nc.finalize()
    return nc


def kernel(branch1, branch2, Wq, bq, Wk, bk, Wv, bv, Wp, bp, **run_kwargs):
    if "nc" not in _CACHE:
        _CACHE["nc"] = _build()
    nc = _CACHE["nc"]

    x1 = np.ascontiguousarray(np.asarray(branch1, np.float32).reshape(B, C, N))
    x2 = np.ascontiguousarray(np.asarray(branch2, np.float32).reshape(B, C, N))
    consts = {
        # pre-transposed: the kernel wants lhsT = W^T layouts
        "wq": np.ascontiguousarray(np.asarray(Wq, np.float32).T),
        "wk": np.ascontiguousarray(np.asarray(Wk, np.float32).T),
        "wv": np.ascontiguousarray(np.asarray(Wv, np.float32).T),
        "wp": np.ascontiguousarray(np.asarray(Wp, np.float32).T),
        "bq": np.ascontiguousarray(bq, np.float32),
        "bk": np.ascontiguousarray(bk, np.float32),
        "bv": np.ascontiguousarray(bv, np.float32),
        "bp": np.ascontiguousarray(bp, np.float32),
    }
    in_maps = []
    for core in range(NCORES):
        b, s = divmod(core, NCORES // B)
        in_maps.append({
            "xq": np.ascontiguousarray(x1[b, :, s * QSH:(s + 1) * QSH]),
            "xkv2": x2[b],
            **consts,
        })
    res = run_bass_kernel_spmd(nc, in_maps, core_ids=list(range(NCORES)),
                               **run_kwargs)
    out = np.empty((B, C, N), np.float32)
    for core in range(NCORES):
        b, s = divmod(core, NCORES // B)
        out[b, :, s * QSH:(s + 1) * QSH] = res.results[core]["out"]
    if run_kwargs:
        _CACHE["last_result"] = res
    return out.reshape(B, C, D, H, W)


# revision 33
# speedup vs baseline: 1.0192x; 1.0192x over previous
"""Trainium2 Bass kernel for EnhancedCrossAttention3D.

Computes, per batch b:
    q = Wq @ x1 + bq            (x1 = branch1[b] reshaped [C, N])
    k = Wk @ x2 + bk
    v = Wv @ x2 + bv
    attn = softmax((q^T k) / sqrt(C), axis=keys)
    out = Wp @ (attn @ v^T)^T + bp      -> [C, N]

Sharding: 8 cores = 2 batches x 4 query shards of 2048. Each core gets its
full K/V source (branch2[b]) and its query shard; no collectives.

On-core algorithm (flash-style, S^T layout):
    S^T[m, n] = sum_c k[c, m] * qT[c, n]   (m = key index on partitions)
    E = exp(S^T / 8)                       (logits are tiny; no max-sub needed)
    PV[c, n]  = sum_m [v | 1][m, c] * E[m, n]   -> row 64 is the softmax denom
    out[o, n] = Wp @ (PV[0:64] / denom + bv) + bp
(bv is folded in after normalization: attn rows sum to 1.)

Structure notes (this PE is clocked at 1.2 GHz; ~1.35us of exp per key chunk
on the Scalar engine is the pacing budget):
  * queries run in two 1024-wide half-passes so the PV accumulator needs only
    2 PSUM banks, freeing 6 banks for a triple-buffered S^T pipeline;
  * S^T matmuls are row-packed: k and qT live in both partition halves and
    two K=64 matmuls run concurrently in opposite halves of the PE array;
  * S/PV matmul operands are bf16 (full-rate streaming + fast weight load),
    PSUM accumulation stays fp32;
  * each pass's epilogue first evacuates the accumulator to SBUF (so the next
    pass can reuse the banks ~1us later) and its PE work is emitted a few
    chunks into the next pass to keep the in-order PE queue from stalling.
"""

import numpy as np
from contextlib import ExitStack

import concourse.bass as bass
import concourse.mybir as mybir
import concourse.tile as tile
from concourse import bacc
from concourse.bass import ts
from concourse.bass_utils import run_bass_kernel_spmd

B, C, D, H, W = 2, 64, 8, 32, 32
N = D * H * W              # 8192 keys per batch
NCORES = 8
QSH = (B * N) // NCORES    # 2048 queries per core
MCH = N // 128             # 64 key chunks of 128
NH = QSH // 1024           # 2 query half-passes
F32 = mybir.dt.float32
F32R = mybir.dt.float32r
BF16 = mybir.dt.bfloat16
AF = mybir.ActivationFunctionType
ALU = mybir.AluOpType

_CACHE = {}


def _emit(tc, xq, xkv2, wq, wk, wv, wp, bq, bk, bv, bp, out):
    nc = tc.nc
    ctx = ExitStack()
    # bf16/f32r writes below intentionally round fp32; errors wash out in the
    # 8192-term attention sums and sit ~1e-4 of output scale.
    ctx.enter_context(nc.allow_low_precision(reason="bf16 attention operands"))
    const = ctx.enter_context(tc.tile_pool(name="const", bufs=1))
    big = ctx.enter_context(tc.tile_pool(name="big", bufs=1))
    ps3 = ctx.enter_context(tc.tile_pool(name="ps3", bufs=3, space="PSUM"))
    acc_p = ctx.enter_context(tc.tile_pool(name="acc", bufs=1, space="PSUM"))
    ex_pool = ctx.enter_context(tc.tile_pool(name="ex", bufs=18))
    small = ctx.enter_context(tc.tile_pool(name="small", bufs=2))

    # ---- loads (gpsimd DMAs cast f32 -> bf16 in flight) ----
    xq_bf = big.tile([C, QSH], BF16)
    for s in range(2):
        nc.gpsimd.dma_start(out=xq_bf[:, ts(s, QSH // 2)],
                            in_=xq[:, ts(s, QSH // 2)])
    xkv_bf = big.tile([C, N], BF16)
    for s in range(8):
        nc.gpsimd.dma_start(out=xkv_bf[:, ts(s, N // 8)],
                            in_=xkv2[:, ts(s, N // 8)])
    # weights arrive pre-transposed from the host (contiguous DMAs), f32 on
    # the otherwise-idle sync queue, tiny casts on DVE
    wqT_f = const.tile([C, C], F32)
    nc.sync.dma_start(out=wqT_f, in_=wq)
    wqT = const.tile([C, C], BF16)
    nc.vector.tensor_copy(wqT, wqT_f)
    wkT_f = const.tile([C, C], F32)
    nc.sync.dma_start(out=wkT_f, in_=wk)
    wkT = const.tile([C, C], BF16)
    nc.vector.tensor_copy(wkT, wkT_f)
    wvT_f = const.tile([C, C], F32)
    nc.sync.dma_start(out=wvT_f, in_=wv)
    wvT = const.tile([C, C], BF16)
    nc.vector.tensor_copy(wvT, wvT_f)
    # projection weight with bp as a 65th row: out = [Wp^T; bp]^T @ [pvn; 1]
    wpT = const.tile([C + 1, C], F32R)
    nc.sync.dma_start(out=wpT[0:C, :], in_=wp)
    nc.sync.dma_start(out=wpT[C:C + 1, :],
                      in_=bp.rearrange("(one c) -> one c", one=1))
    bq_sb = const.tile([C, 1], F32)
    nc.sync.dma_start(out=bq_sb, in_=bq.rearrange("(c one) -> c one", one=1))
    bk_sb = const.tile([C, 1], F32)
    nc.sync.dma_start(out=bk_sb, in_=bk.rearrange("(c one) -> c one", one=1))
    bv_sb = const.tile([C, 1], F32)
    nc.sync.dma_start(out=bv_sb, in_=bv.rearrange("(c one) -> c one", one=1))
    ones_f32 = const.tile([128, MCH], F32)
    nc.vector.memset(ones_f32, 1.0)

    # ---- q/k projections ----
    # Both live twice (partitions 0-63 and 64-127) so the S^T matmuls can be
    # row-packed; bias-add on DVE (lower half), bf16 fast-copy for the upper.
    qT_sb = big.tile([128, QSH], BF16)
    k_sb = big.tile([128, N], BF16)
    NKT = N // 512

    def q_proj(t):
        pq = ps3.tile([128, 1024], F32, tag="ps")
        nc.tensor.matmul(pq[0:C, 0:512], lhsT=wqT, rhs=xq_bf[:, ts(t, 512)],
                         start=True, stop=True)
        nc.vector.tensor_scalar_add(qT_sb[0:C, ts(t, 512)], pq[0:C, 0:512],
                                    bq_sb)
        nc.vector.tensor_copy(qT_sb[C:2 * C, ts(t, 512)],
                              qT_sb[0:C, ts(t, 512)])

    def k_proj(t):
        # projects k columns t*512..(t+1)*512 (covers key chunks 4t..4t+3)
        pk = ps3.tile([128, 1024], F32, tag="ps")
        nc.tensor.matmul(pk[0:C, 0:512], lhsT=wkT, rhs=xkv_bf[:, ts(t, 512)],
                         start=True, stop=True)
        nc.vector.tensor_scalar_add(k_sb[0:C, ts(t, 512)], pk[0:C, 0:512],
                                    bk_sb)
        nc.vector.tensor_copy(k_sb[C:2 * C, ts(t, 512)], k_sb[0:C, ts(t, 512)])

    # pass 0 needs q tiles 0-1 and k tile 0 before S(0) can go
    q_proj(0)
    q_proj(1)
    k_proj(0)
    q_proj(2)
    q_proj(3)
    k_proj(1)
    k_done = 2
    # v[m, c] with a ones column (row 64 of PV becomes the softmax denominator)
    v_sb = big.tile([128, MCH, C + 1], BF16)
    nc.vector.tensor_copy(v_sb[:, :, C], ones_f32)

    def epilogue_dve(acc):
        """Normalize the accumulator (+bv) off-PE; returns the pvn tiles."""
        denom = small.tile([1, 1024], F32, tag="denom")
        nc.vector.tensor_copy(denom, acc[C:C + 1, :])
        rbd = small.tile([C, 1024], F32, tag="rbd")
        nc.gpsimd.partition_broadcast(rbd, denom)
        rb = small.tile([C, 1024], F32, tag="rb")
        nc.vector.reciprocal(rb, rbd)
        pvns = []
        for t in range(2):
            pvn = small.tile([C + 1, 512], F32R, tag="pvn")
            nc.vector.memset(pvn.bitcast(F32)[C:C + 1, :], 1.0)
            nc.vector.tensor_mul(pvn[0:C, :], acc[0:C, ts(t, 512)],
                                 rb[:, ts(t, 512)])
            nc.vector.tensor_scalar_add(pvn[0:C, :], pvn[0:C, :], bv_sb)
            pvns.append(pvn)
        return pvns

    def epilogue_pe(p, pvns):
        """Project the normalized PV and store out[:, p*1024:(p+1)*1024]."""
        for t in range(2):
            pp = ps3.tile([128, 1024], F32, tag="ps")
            nc.tensor.matmul(pp[0:C, 0:512], lhsT=wpT, rhs=pvns[t],
                             start=True, stop=True)
            o_sb = small.tile([C, 512], F32, tag="o")
            nc.vector.tensor_copy(o_sb, pp[0:C, 0:512])
            nc.sync.dma_start(out=out[:, ts(p * 2 + t, 512)], in_=o_sb)

    # ---- flash loop: two query half-passes over all key chunks ----
    # PV trails S/exp by LAG chunks (ex tiles buffer the gap) so that at a
    # pass boundary the next pass's S/exp pipeline runs while the previous
    # accumulator is still being normalized (PV's WAR wait then doesn't
    # stall the in-order PE queue).
    LAG = 16
    pending = None
    exq = {}
    acc = None
    for u in range(NH * MCH + LAG):
        if u < NH * MCH:
            p, i = divmod(u, MCH)
            if p == 0 and i % 4 == 0 and k_done < NKT:
                # produce k two tiles (8 chunks) ahead of consumption
                k_proj(k_done)
                k_done += 1
            if p == 0 and i % 4 == 0:
                # v chunks i..i+3: 4 matmuls into one psum slot, 1 cast
                g = i // 4
                pv = ps3.tile([128, 1024], F32, tag="ps")
                for j in range(4):
                    nc.tensor.matmul(pv[:, ts(j, C)],
                                     lhsT=xkv_bf[:, ts(4 * g + j, 128)],
                                     rhs=wvT, start=True, stop=True)
                nc.vector.tensor_copy(
                    v_sb[:, 4 * g:4 * g + 4, 0:C],
                    pv[:, 0:4 * C].rearrange("p (g c) -> p g c", c=C))
            if pending is not None and u == MCH + 20:
                epilogue_pe(*pending)
                pending = None
            s_ps = ps3.tile([128, 1024], F32, tag="ps")
            for sub in range(2):
                lo = C * sub
                nc.tensor.matmul(s_ps[:, ts(sub, 512)],
                                 lhsT=k_sb[lo:lo + C, ts(i, 128)],
                                 rhs=qT_sb[lo:lo + C,
                                           p * 1024 + 512 * sub:
                                           p * 1024 + 512 * (sub + 1)],
                                 start=True, stop=True)
            ex = ex_pool.tile([128, 1024], BF16)
            nc.scalar.activation(ex, s_ps, AF.Exp, scale=0.125)
            exq[u] = ex
        j = u - LAG
        if j >= 0 and j < NH * MCH:
            jp, ji = divmod(j, MCH)
            if ji == 0:
                acc = acc_p.tile([C + 1, 1024], F32, tag="acc")
            ex = exq.pop(j)
            for sub in range(2):
                nc.tensor.matmul(acc[:, ts(sub, 512)], lhsT=v_sb[:, ji, :],
                                 rhs=ex[:, ts(sub, 512)],
                                 start=(ji == 0), stop=(ji == MCH - 1),
                                 skip_group_check=True)
            if ji == MCH - 1:
                pvns = epilogue_dve(acc)
                if jp < NH - 1:
                    pending = (jp, pvns)
                else:
                    epilogue_pe(jp, pvns)
    ctx.close()


def _build():
    nc = bacc.Bacc("TRN2", target_bir_lowering=False, debug=False,
                   num_devices=NCORES)
    aps = {}
    aps["xq"] = nc.dram_tensor("xq", [C, QSH], F32, kind="ExternalInput").ap()
    aps["xkv2"] = nc.dram_tensor("xkv2", [C, N], F32, kind="ExternalInput").ap()
    for nm in ("wq", "wk", "wv"):
        aps[nm] = nc.dram_tensor(nm, [C, C], F32, kind="ExternalInput").ap()
    aps["wp"] = nc.dram_tensor("wp", [C, C], F32R, kind="ExternalInput").ap()
    for nm in ("bq", "bk", "bv"):
        aps[nm] = nc.dram_tensor(nm, [C], F32, kind="ExternalInput").ap()
    aps["bp"] = nc.dram_tensor("bp", [C], F32R, kind="ExternalInput").ap()
    aps["out"] = nc.dram_tensor("out", [C, QSH], F32, kind="ExternalOutput").ap()
    with tile.TileContext(nc) as tc:
        _emit(tc, **aps)
    nc.finalize()
    return nc


def kernel(branch1, branch2, Wq, bq, Wk, bk, Wv, bv, Wp, bp, **run_kwargs):
    if "nc" not in _CACHE:
        _CACHE["nc"] = _build()
    nc = _CACHE["nc"]

    x1 = np.ascontiguousarray(np.asarray(branch1, np.float32).reshape(B, C, N))
    x2 = np.ascontiguousarray(np.asarray(branch2, np.float32).reshape(B, C, N))
    consts = {
        # pre-transposed: the kernel wants lhsT = W^T layouts
        "wq": np.ascontiguousarray(np.asarray(Wq, np.float32).T),
        "wk": np.ascontiguousarray(np.asarray(Wk, np.float32).T),
        "wv": np.ascontiguousarray(np.asarray(Wv, np.float32).T),
        "wp": np.ascontiguousarray(np.asarray(Wp, np.float32).T),
        "bq": np.ascontiguousarray(bq, np.float32),
        "bk": np.ascontiguousarray(bk, np.float32),
        "bv": np.ascontiguousarray(bv, np.float32),
        "bp": np.ascontiguousarray(bp, np.float32),
    }
    in_maps = []
    for core in range(NCORES):
        b, s = divmod(core, NCORES // B)
        in_maps.append({
            "xq": np.ascontiguousarray(x1[b, :, s * QSH:(s + 1) * QSH]),
            "xkv2": x2[b],
            **consts,
        })
    res = run_bass_kernel_spmd(nc, in_maps, core_ids=list(range(NCORES)),
                               **run_kwargs)
    out = np.empty((B, C, N), np.float32)
    for core in range(NCORES):
        b, s = divmod(core, NCORES // B)
        out[b, :, s * QSH:(s + 1) * QSH] = res.results[core]["out"]
    if run_kwargs:
        _CACHE["last_result"] = res
    return out.reshape(B, C, D, H, W)
